# revision 8
# baseline (speedup 1.0000x reference)
"""GBST embedding kernel for Trainium2, data-parallel over batch on 8 cores.

Even/odd reformulation: with t the output index (T = L/2 after the final
DS=2 mean-pool), the whole softmax-weighted candidate combine collapses to

    o[t] = sum_{d=-2..3} w_d[t] * y[2t+d]

a 6-diagonal banded product on the t grid.  Splitting y into parity planes
yE[t]=y[2t], yO[t]=y[2t+1] makes every shifted view contiguous.

Per core (one batch element, [d_chunk, t] layout):
- Embedding gather folded into the conv: G_k = emb @ conv_w[:,:,k].T (bf16),
  y parity tiles computed as 10-matmul PSUM groups against even/odd one-hots.
- Scores s1 via scw matmuls on y tiles; softmax + w_d rows computed in
  [32-partition, 32] planes per 1024-t chunk with host-precomputed masks.
- Combine: 6 contiguous muls + 5 adds per (chunk, dc), split DVE/GPSIMD.
- PE transposes [d, t] -> [t, d] for contiguous output stores.
"""
import sys
sys.path.insert(0, "/opt/trn_rl_repo")
import numpy as np
import ml_dtypes

import concourse.bass as bass
import concourse.bacc as bacc
import concourse.tile as tile
from concourse import mybir
from concourse.bass_utils import run_bass_kernel_spmd

bf16 = ml_dtypes.bfloat16
F32 = mybir.dt.float32
BF = mybir.dt.bfloat16
OP = mybir.AluOpType
AF = mybir.ActivationFunctionType

L, T, V, D, K = 8192, 4096, 256, 512, 5
NDC, NVC = 4, 2
TT_ = 512            # conv t-tile width
NTI = T // TT_       # 8 tiles
TSEG = 1024          # chunk width in t
NCH = 4
YW = 4104            # y/oh tile width (col = t+1, t in [-1, 4103))
SW = 4104            # s1 row width
JW = 2736            # s3sum row width

TRACE = False
LAST_RESULT = None
_NC = None

# taps: (k, src_parity, col_offset_from_tile_base) for output parity E and O
TAPS = {
    "E": [(0, "E", 0), (1, "O", 0), (2, "E", 1), (3, "O", 1), (4, "E", 2)],
    "O": [(0, "O", 0), (1, "E", 1), (2, "O", 1), (3, "E", 2), (4, "O", 2)],
}
# chunk c is emitted after conv tile TRIG[c]
TRIG = {2: 0, 4: 1, 6: 2, 7: 3}


def _ceil_div(a, b):
    return -(-a // b)


def _build():
    nc = bacc.Bacc("TRN2", target_bir_lowering=False)
    idse_d = nc.dram_tensor("idse", [1, YW], BF, kind="ExternalInput")
    idso_d = nc.dram_tensor("idso", [1, YW], BF, kind="ExternalInput")
    gws_d = nc.dram_tensor("gws", [128, 40 * 128], BF, kind="ExternalInput")
    iot_d = nc.dram_tensor("iot", [128, 2], F32, kind="ExternalInput")
    scw_d = nc.dram_tensor("scw", [128, 4], BF, kind="ExternalInput")
    bias_d = nc.dram_tensor("bias", [128, 4], F32, kind="ExternalInput")
    ident_d = nc.dram_tensor("ident", [128, 128], BF, kind="ExternalInput")
    mask_d = nc.dram_tensor("mask", [128, 8 * 32], BF, kind="ExternalInput")
    out_d = nc.dram_tensor("out", [T, D], F32, kind="ExternalOutput")
    # DRAM staging for the 6 w_delta rows (broadcast source)
    wrow_d = [nc.dram_tensor(f"wrow{di}", [1, T], BF) for di in range(6)]

    with tile.TileContext(nc) as tc:
        with tc.tile_pool(name="const", bufs=1) as cst, \
             tc.tile_pool(name="persist", bufs=1) as per, \
             tc.tile_pool(name="rows", bufs=1) as rws, \
             tc.tile_pool(name="plane", bufs=1) as pln, \
             tc.tile_pool(name="wseg", bufs=2) as wsg, \
             tc.tile_pool(name="ctp", bufs=2) as ctp, \
             tc.tile_pool(name="accp", bufs=2) as accp, \
             tc.tile_pool(name="otp", bufs=2) as otp, \
             tc.tile_pool(name="psA", bufs=3, space="PSUM") as psA, \
             tc.tile_pool(name="psB", bufs=2, space="PSUM") as psB, \
             tc.tile_pool(name="psT", bufs=2, space="PSUM") as psT:

            # ---- constants
            gws_t = cst.tile([128, 40 * 128], BF)
            nc.sync.dma_start(out=gws_t[:], in_=gws_d[:])
            iot_t = cst.tile([128, 2], F32)
            nc.sync.dma_start(out=iot_t[:], in_=iot_d[:])
            scw_t = cst.tile([128, 4], BF)
            nc.sync.dma_start(out=scw_t[:], in_=scw_d[:])
            bias_t = cst.tile([128, 4], F32)
            nc.sync.dma_start(out=bias_t[:], in_=bias_d[:])
            ident_t = cst.tile([128, 128], BF)
            nc.sync.dma_start(out=ident_t[:], in_=ident_d[:])
            mask_t = cst.tile([128, 8 * 32], BF)
            nc.sync.dma_start(out=mask_t[:], in_=mask_d[:])
            MODD, MEVEN, M3E0, M3E1, M3E2, M3N0, M3N1, M3N2 = range(8)

            def mk(i):
                return mask_t[:, i * 32:(i + 1) * 32]

            # ---- persistent tensors
            ids_b = per.tile([128, YW], BF, name="idsb", tag="idsb")
            oh = {p: [per.tile([128, YW], BF, name=f"oh{p}{vc}", tag=f"oh{p}{vc}")
                      for vc in range(NVC)] for p in "EO"}
            for p, src_d in (("E", idse_d), ("O", idso_d)):
                nc.sync.dma_start(out=ids_b[:],
                                  in_=src_d[0:1, :].partition_broadcast(128))
                for vc in range(NVC):
                    nc.vector.tensor_scalar(
                        out=oh[p][vc][:], in0=ids_b[:],
                        scalar1=iot_t[:, vc:vc + 1], scalar2=None,
                        op0=OP.is_equal)

            y = {p: [per.tile([128, YW], BF, name=f"y{p}{dc}", tag=f"y{p}{dc}")
                     for dc in range(NDC)] for p in "EO"}
            for p in "EO":
                for dc in range(NDC):
                    nc.vector.memset(y[p][dc][:, 0:1], 0.0)
                    nc.vector.memset(y[p][dc][:, T + 1:YW], 0.0)
            s1 = {p: rws.tile([1, SW], BF, name=f"s1{p}", tag=f"s1{p}")
                  for p in "EO"}
            for p in "EO":
                nc.vector.memset(s1[p][0:1, T:SW], 0.0)
            s3sum = rws.tile([1, JW], BF, name="s3sum", tag="s3sum")
            us3r = {p: rws.tile([1, T + 8], BF, name=f"us3{p}", tag=f"us3{p}")
                    for p in "EO"}

            ov = out_d[:].rearrange("(tb p) (dc c) -> p tb dc c", p=128, c=128)

            def emit_weights(c):
                """softmax + w_delta rows for chunk c (t in [1024c, 1024c+1024))."""
                t0 = c * TSEG
                t1 = t0 + TSEG
                P0, P1 = 32 * c, 32 * (c + 1)  # plane partition range

                # --- s3sum rows (strided, f32, DVE)
                jlo = (2 * t0) // 3
                jhi = (2 * t1 - 1) // 3
                # even j = 2m
                m0 = _ceil_div(jlo, 2)
                nme = jhi // 2 - m0 + 1
                nc.vector.tensor_tensor(
                    out=s3sum[0:1, 2 * m0:2 * m0 + 2 * nme:2],
                    in0=s1["E"][0:1, 3 * m0:3 * m0 + 3 * nme:3],
                    in1=s1["O"][0:1, 3 * m0:3 * m0 + 3 * nme:3], op=OP.add)
                nc.vector.tensor_tensor(
                    out=s3sum[0:1, 2 * m0:2 * m0 + 2 * nme:2],
                    in0=s3sum[0:1, 2 * m0:2 * m0 + 2 * nme:2],
                    in1=s1["E"][0:1, 3 * m0 + 1:3 * m0 + 1 + 3 * nme:3], op=OP.add)
                # odd j = 2m+1
                mo0 = _ceil_div(jlo - 1, 2)
                nmo = (jhi - 1) // 2 - mo0 + 1
                nc.vector.tensor_tensor(
                    out=s3sum[0:1, 2 * mo0 + 1:2 * mo0 + 1 + 2 * nmo:2],
                    in0=s1["O"][0:1, 3 * mo0 + 1:3 * mo0 + 1 + 3 * nmo:3],
                    in1=s1["E"][0:1, 3 * mo0 + 2:3 * mo0 + 2 + 3 * nmo:3], op=OP.add)
                nc.vector.tensor_tensor(
                    out=s3sum[0:1, 2 * mo0 + 1:2 * mo0 + 1 + 2 * nmo:2],
                    in0=s3sum[0:1, 2 * mo0 + 1:2 * mo0 + 1 + 2 * nmo:2],
                    in1=s1["O"][0:1, 3 * mo0 + 2:3 * mo0 + 2 + 3 * nmo:3], op=OP.add)

                # --- us3 rows: upsample s3sum/3 to the t grid (Scalar, strided)
                for side, jadd in (("E", (0, 0, 1)), ("O", (0, 1, 1))):
                    for rho in range(3):
                        ts = t0 + ((rho - t0) % 3)
                        n = _ceil_div(t1 - ts, 3)
                        mm0 = (ts - rho) // 3
                        j0 = 2 * mm0 + jadd[rho]
                        nc.scalar.activation(
                            out=us3r[side][0:1, ts:ts + 3 * n:3],
                            in_=s3sum[0:1, j0:j0 + 2 * n:2],
                            func=AF.Copy, scale=1.0 / 3.0)

                # --- softmax planes [32, 32] on partitions [P0:P1]
                LE = pln.tile([128, 32], BF, tag="LE", name=f"LE{c}")
                LO = pln.tile([128, 32], BF, tag="LO", name=f"LO{c}")
                L3a = pln.tile([128, 32], BF, tag="L3a", name=f"L3a{c}")
                L3b = pln.tile([128, 32], BF, tag="L3b", name=f"L3b{c}")
                nc.sync.dma_start(out=LE[P0:P1, :], in_=s1["E"][0:1, t0:t1])
                nc.sync.dma_start(out=LO[P0:P1, :], in_=s1["O"][0:1, t0:t1])
                nc.sync.dma_start(out=L3a[P0:P1, :], in_=us3r["E"][0:1, t0:t1])
                nc.sync.dma_start(out=L3b[P0:P1, :], in_=us3r["O"][0:1, t0:t1])
                L2 = pln.tile([128, 32], F32, tag="L2", name=f"L2{c}")
                nc.vector.tensor_tensor(out=L2[P0:P1, :], in0=LE[P0:P1, :],
                                        in1=LO[P0:P1, :], op=OP.add)
                nc.vector.tensor_scalar(out=L2[P0:P1, :], in0=L2[P0:P1, :],
                                        scalar1=0.5, scalar2=None, op0=OP.mult)
                L4 = pln.tile([128, 32], F32, tag="L4", name=f"L4{c}")
                L2pair = L2[P0:P1, :].rearrange("p (n two) -> p n two", two=2)
                nc.vector.tensor_tensor(out=L4[P0:P1, 0:16], in0=L2pair[:, :, 0],
                                        in1=L2pair[:, :, 1], op=OP.add)
                U4 = pln.tile([128, 32], F32, tag="U4", name=f"U4{c}")
                nc.vector.tensor_scalar(
                    out=U4[P0:P1, :].rearrange("p (n two) -> p n two", two=2),
                    in0=L4[P0:P1, 0:16].unsqueeze(2).to_broadcast([P1 - P0, 16, 2]),
                    scalar1=0.5, scalar2=None, op0=OP.mult)

                X = pln.tile([128, 256], BF, tag="X", name=f"X{c}")
                E = pln.tile([128, 256], BF, tag="E", name=f"E{c}")
                for half, (l1, l3) in enumerate(((LE, L3a), (LO, L3b))):
                    mx = pln.tile([128, 32], F32, tag=f"mx{half}",
                                  name=f"mx{half}_{c}")
                    nc.vector.tensor_tensor(out=mx[P0:P1, :], in0=l1[P0:P1, :],
                                            in1=L2[P0:P1, :], op=OP.max)
                    nc.vector.tensor_tensor(out=mx[P0:P1, :], in0=mx[P0:P1, :],
                                            in1=l3[P0:P1, :], op=OP.max)
                    nc.vector.tensor_tensor(out=mx[P0:P1, :], in0=mx[P0:P1, :],
                                            in1=U4[P0:P1, :], op=OP.max)
                    for li, lt in enumerate((l1, L2, l3, U4)):
                        o0 = half * 128 + li * 32
                        nc.vector.tensor_tensor(
                            out=X[P0:P1, o0:o0 + 32], in0=lt[P0:P1, :],
                            in1=mx[P0:P1, :], op=OP.subtract)
                nc.scalar.activation(out=E[P0:P1, :], in_=X[P0:P1, :], func=AF.Exp)

                W = {}
                for half, pname in enumerate("EO"):
                    h0 = half * 128
                    Z = pln.tile([128, 32], F32, tag=f"Z{half}",
                                 name=f"Z{half}_{c}")
                    nc.vector.tensor_tensor(out=Z[P0:P1, :],
                                            in0=E[P0:P1, h0:h0 + 32],
                                            in1=E[P0:P1, h0 + 32:h0 + 64], op=OP.add)
                    nc.vector.tensor_tensor(out=Z[P0:P1, :], in0=Z[P0:P1, :],
                                            in1=E[P0:P1, h0 + 64:h0 + 96], op=OP.add)
                    nc.vector.tensor_tensor(out=Z[P0:P1, :], in0=Z[P0:P1, :],
                                            in1=E[P0:P1, h0 + 96:h0 + 128], op=OP.add)
                    R = pln.tile([128, 32], F32, tag=f"R{half}",
                                 name=f"R{half}_{c}")
                    nc.vector.reciprocal(out=R[P0:P1, :], in_=Z[P0:P1, :])
                    # R' = 0.5/Z
                    nc.vector.tensor_scalar(out=R[P0:P1, :], in0=R[P0:P1, :],
                                            scalar1=0.5, scalar2=None, op0=OP.mult)
                    for mi in range(4):
                        wt = pln.tile([128, 32], F32, tag=f"w{mi}{half}",
                                      name=f"w{mi}{half}_{c}")
                        nc.vector.tensor_tensor(
                            out=wt[P0:P1, :],
                            in0=E[P0:P1, h0 + mi * 32:h0 + mi * 32 + 32],
                            in1=R[P0:P1, :], op=OP.mult)
                        W[(mi, pname)] = wt

                # combined raw sums (already carry the 0.5 from R')
                W2s = pln.tile([128, 32], F32, tag="W2s", name=f"W2s{c}")
                nc.vector.tensor_tensor(out=W2s[P0:P1, :], in0=W[(1, "E")][P0:P1, :],
                                        in1=W[(1, "O")][P0:P1, :], op=OP.add)
                W4s = pln.tile([128, 32], F32, tag="W4s", name=f"W4s{c}")
                nc.vector.tensor_tensor(out=W4s[P0:P1, :], in0=W[(3, "E")][P0:P1, :],
                                        in1=W[(3, "O")][P0:P1, :], op=OP.add)
                P2 = pln.tile([128, 32], F32, tag="P2", name=f"P2{c}")
                nc.vector.tensor_scalar(out=P2[P0:P1, :], in0=W2s[P0:P1, :],
                                        scalar1=0.5, scalar2=None, op0=OP.mult)
                P4d = pln.tile([128, 32], F32, tag="P4d", name=f"P4d{c}")
                nc.vector.tensor_scalar(out=P4d[P0:P1, :], in0=W4s[P0:P1, :],
                                        scalar1=0.25, scalar2=None, op0=OP.mult)
                P4o = pln.tile([128, 32], F32, tag="P4o", name=f"P4o{c}")
                nc.vector.tensor_tensor(out=P4o[P0:P1, :], in0=W4s[P0:P1, :],
                                        in1=mk(MODD)[P0:P1, :], op=OP.mult)
                P4e = pln.tile([128, 32], F32, tag="P4e", name=f"P4e{c}")
                nc.vector.tensor_tensor(out=P4e[P0:P1, :], in0=W4s[P0:P1, :],
                                        in1=mk(MEVEN)[P0:P1, :], op=OP.mult)

                w3E, w3O = W[(2, "E")], W[(2, "O")]
                q = {}
                for nm, src, mi in (("a_r2", w3E, M3E1), ("a_n0", w3E, M3N0),
                                    ("a_n2", w3E, M3N1), ("a_r0", w3E, M3E0),
                                    ("b_e2", w3O, M3E2), ("b_ne1", w3O, M3N1),
                                    ("b_ne2", w3O, M3N2), ("b_e1", w3O, M3E1)):
                    qt = pln.tile([128, 32], F32, tag=f"q{nm}", name=f"q{nm}_{c}")
                    nc.vector.tensor_tensor(out=qt[P0:P1, :], in0=src[P0:P1, :],
                                            in1=mk(mi)[P0:P1, :], op=OP.mult)
                    q[nm] = qt
                qa_d = pln.tile([128, 32], F32, tag="qa_d", name=f"qa_d{c}")
                nc.vector.tensor_scalar(out=qa_d[P0:P1, :], in0=w3E[P0:P1, :],
                                        scalar1=1.0 / 3.0, scalar2=None, op0=OP.mult)
                qb_d = pln.tile([128, 32], F32, tag="qb_d", name=f"qb_d{c}")
                nc.vector.tensor_scalar(out=qb_d[P0:P1, :], in0=w3O[P0:P1, :],
                                        scalar1=1.0 / 3.0, scalar2=None, op0=OP.mult)

                wd = [pln.tile([128, 32], BF, tag=f"wd{di}", name=f"wd{di}_{c}")
                      for di in range(6)]
                # delta=-2: P4o + qa_r2
                nc.vector.tensor_tensor(out=wd[0][P0:P1, :], in0=P4o[P0:P1, :],
                                        in1=q["a_r2"][P0:P1, :], op=OP.add)
                # delta=-1: P4o + qa_n0 + qb_e2
                tmp = pln.tile([128, 32], F32, tag="wtmp", name=f"wtmp{c}")
                nc.vector.tensor_tensor(out=tmp[P0:P1, :], in0=P4o[P0:P1, :],
                                        in1=q["a_n0"][P0:P1, :], op=OP.add)
                nc.vector.tensor_tensor(out=wd[1][P0:P1, :], in0=tmp[P0:P1, :],
                                        in1=q["b_e2"][P0:P1, :], op=OP.add)
                # delta=0: w1E' + P2 + P4d + qa_d + qb_ne1
                t0_ = pln.tile([128, 32], F32, tag="wt0", name=f"wt0_{c}")
                nc.vector.tensor_tensor(out=t0_[P0:P1, :],
                                        in0=W[(0, "E")][P0:P1, :],
                                        in1=P2[P0:P1, :], op=OP.add)
                nc.vector.tensor_tensor(out=t0_[P0:P1, :], in0=t0_[P0:P1, :],
                                        in1=P4d[P0:P1, :], op=OP.add)
                nc.vector.tensor_tensor(out=t0_[P0:P1, :], in0=t0_[P0:P1, :],
                                        in1=qa_d[P0:P1, :], op=OP.add)
                nc.vector.tensor_tensor(out=wd[2][P0:P1, :], in0=t0_[P0:P1, :],
                                        in1=q["b_ne1"][P0:P1, :], op=OP.add)
                # delta=1: w1O' + P2 + P4d + qa_n2 + qb_d
                t1_ = pln.tile([128, 32], F32, tag="wt1", name=f"wt1_{c}")
                nc.vector.tensor_tensor(out=t1_[P0:P1, :],
                                        in0=W[(0, "O")][P0:P1, :],
                                        in1=P2[P0:P1, :], op=OP.add)
                nc.vector.tensor_tensor(out=t1_[P0:P1, :], in0=t1_[P0:P1, :],
                                        in1=P4d[P0:P1, :], op=OP.add)
                nc.vector.tensor_tensor(out=t1_[P0:P1, :], in0=t1_[P0:P1, :],
                                        in1=q["a_n2"][P0:P1, :], op=OP.add)
                nc.vector.tensor_tensor(out=wd[3][P0:P1, :], in0=t1_[P0:P1, :],
                                        in1=qb_d[P0:P1, :], op=OP.add)
                # delta=2: P4e + qa_r0 + qb_ne2
                tmp2 = pln.tile([128, 32], F32, tag="wtmp2", name=f"wtmp2{c}")
                nc.vector.tensor_tensor(out=tmp2[P0:P1, :], in0=P4e[P0:P1, :],
                                        in1=q["a_r0"][P0:P1, :], op=OP.add)
                nc.vector.tensor_tensor(out=wd[4][P0:P1, :], in0=tmp2[P0:P1, :],
                                        in1=q["b_ne2"][P0:P1, :], op=OP.add)
                # delta=3: P4e + qb_e1
                nc.vector.tensor_tensor(out=wd[5][P0:P1, :], in0=P4e[P0:P1, :],
                                        in1=q["b_e1"][P0:P1, :], op=OP.add)

                for di in range(6):
                    nc.sync.dma_start(out=wrow_d[di][0:1, t0:t1],
                                      in_=wd[di][P0:P1, :])

            def emit_combine(c):
                t0 = c * TSEG
                wb = []
                for di in range(6):
                    wt = wsg.tile([128, TSEG], BF, tag=f"wb{di}",
                                  name=f"wb{di}_{c}")
                    nc.sync.dma_start(
                        out=wt[:],
                        in_=wrow_d[di][0:1, t0:t0 + TSEG].partition_broadcast(128))
                    wb.append(wt)
                for dc in range(NDC):
                    yE, yO = y["E"][dc], y["O"][dc]
                    # views: col = t+1
                    vEm2 = yE[:, t0:t0 + TSEG]          # yE[t-1], aligned
                    vE0 = yE[:, t0 + 1:t0 + 1 + TSEG]   # yE[t], misaligned
                    vE2 = yE[:, t0 + 2:t0 + 2 + TSEG]   # yE[t+1], aligned
                    vOm1 = yO[:, t0:t0 + TSEG]
                    vO1 = yO[:, t0 + 1:t0 + 1 + TSEG]
                    vO3 = yO[:, t0 + 2:t0 + 2 + TSEG]
                    # DVE chain: 4 aligned muls folded with adds
                    accD = ctp.tile([128, TSEG], BF, tag="accD",
                                    name=f"accD_{c}_{dc}")
                    tD = ctp.tile([128, TSEG], BF, tag="tD", name=f"tD_{c}_{dc}")
                    nc.vector.tensor_tensor(out=accD[:], in0=vEm2, in1=wb[0][:],
                                            op=OP.mult)
                    nc.vector.tensor_tensor(out=tD[:], in0=vOm1, in1=wb[1][:],
                                            op=OP.mult)
                    nc.vector.tensor_tensor(out=accD[:], in0=accD[:], in1=tD[:],
                                            op=OP.add)
                    nc.vector.tensor_tensor(out=tD[:], in0=vE2, in1=wb[4][:],
                                            op=OP.mult)
                    nc.vector.tensor_tensor(out=accD[:], in0=accD[:], in1=tD[:],
                                            op=OP.add)
                    nc.vector.tensor_tensor(out=tD[:], in0=vO3, in1=wb[5][:],
                                            op=OP.mult)
                    nc.vector.tensor_tensor(out=accD[:], in0=accD[:], in1=tD[:],
                                            op=OP.add)
                    # GPSIMD chain: 2 misaligned muls + 1 add
                    accG = ctp.tile([128, TSEG], BF, tag="accG",
                                    name=f"accG_{c}_{dc}")
                    tG = ctp.tile([128, TSEG], BF, tag="tG", name=f"tG_{c}_{dc}")
                    nc.gpsimd.tensor_tensor(out=accG[:], in0=vE0, in1=wb[2][:],
                                            op=OP.mult)
                    nc.gpsimd.tensor_tensor(out=tG[:], in0=vO1, in1=wb[3][:],
                                            op=OP.mult)
                    nc.gpsimd.tensor_tensor(out=accG[:], in0=accG[:], in1=tG[:],
                                            op=OP.add)
                    acc = accp.tile([128, TSEG], BF, tag="acc",
                                    name=f"acc_{c}_{dc}")
                    nc.vector.tensor_tensor(out=acc[:], in0=accD[:], in1=accG[:],
                                            op=OP.add)
                    # transpose [d, t] -> [t, d] and store
                    for q4 in range(2):
                        pt = psT.tile([128, 512], BF, tag="tp",
                                      name=f"pt_{c}_{dc}_{q4}")
                        for qq in range(4):
                            nc.tensor.transpose(
                                out=pt[:, qq * 128:(qq + 1) * 128],
                                in_=acc[:, (q4 * 4 + qq) * 128:
                                        (q4 * 4 + qq + 1) * 128],
                                identity=ident_t[:])
                        ot = otp.tile([128, 512], F32, tag="ot",
                                      name=f"ot_{c}_{dc}_{q4}")
                        nc.scalar.copy(out=ot[:], in_=pt[:])
                        tb0 = c * 8 + q4 * 4
                        nc.sync.dma_start(
                            out=ov[:, tb0:tb0 + 4, dc, :],
                            in_=ot[:].rearrange("p (tb c) -> p tb c", c=128))

            # ---- main conv loop
            for i in range(NTI):
                col0 = i * TT_
                for p in "EO":
                    for dc in range(NDC):
                        ps = psA.tile([128, TT_], F32, tag="convps",
                                      name=f"ps_{i}_{p}_{dc}")
                        for j, (k, src, off) in enumerate(TAPS[p]):
                            for vc in range(NVC):
                                jj = j * 2 + vc
                                nc.tensor.matmul(
                                    out=ps[:],
                                    lhsT=gws_t[:, ((k * 2 + vc) * 4 + dc) * 128:
                                               ((k * 2 + vc) * 4 + dc) * 128 + 128],
                                    rhs=oh[src][vc][:, col0 + off:
                                                    col0 + off + TT_],
                                    start=(jj == 0), stop=(jj == 9))
                        nc.scalar.activation(
                            out=y[p][dc][:, col0 + 1:col0 + 1 + TT_], in_=ps[:],
                            func=AF.Identity, bias=bias_t[:, dc:dc + 1])
                    ps1 = psB.tile([1, TT_], F32, tag="s1ps", name=f"ps1_{i}_{p}")
                    for dc in range(NDC):
                        nc.tensor.matmul(out=ps1[:], lhsT=scw_t[:, dc:dc + 1],
                                         rhs=y[p][dc][:, col0 + 1:col0 + 1 + TT_],
                                         start=(dc == 0), stop=(dc == NDC - 1))
                    nc.scalar.copy(out=s1[p][0:1, col0:col0 + TT_], in_=ps1[:])
                if i in TRIG:
                    c = TRIG[i]
                    emit_weights(c)
                    emit_combine(c)
    nc.compile()
    return nc


def _get_nc():
    global _NC
    if _NC is None:
        _NC = _build()
    return _NC


def _host_inputs(input_ids, emb, conv_w, conv_b, score_w):
    G = np.einsum("oik,vi->kvo", conv_w.astype(np.float64),
                  emb.astype(np.float64)).astype(np.float32)  # [K, V, D]
    gws = np.zeros((128, 40, 128), np.float32)
    for k in range(K):
        for vc in range(NVC):
            for dc in range(NDC):
                gws[:, (k * 2 + vc) * 4 + dc, :] = \
                    G[k, vc * 128:(vc + 1) * 128, dc * 128:(dc + 1) * 128]
    gws = gws.reshape(128, 40 * 128).astype(bf16)
    iot = np.stack([np.arange(128), np.arange(128) + 128], axis=1).astype(np.float32)
    scw = score_w.reshape(4, 128).T.astype(bf16)
    biasm = conv_b.reshape(4, 128).T.astype(np.float32)
    ident = np.eye(128, dtype=np.float32).astype(bf16)

    t = (np.arange(128)[:, None] * 32 + np.arange(32)[None, :])  # [128, 32]
    masks = np.zeros((128, 8, 32), np.float32)
    masks[:, 0] = 0.25 * (t % 2 == 1)
    masks[:, 1] = 0.25 * (t % 2 == 0)
    masks[:, 2] = (1.0 / 3.0) * (t % 3 == 0)
    masks[:, 3] = (1.0 / 3.0) * (t % 3 == 1)
    masks[:, 4] = (1.0 / 3.0) * (t % 3 == 2)
    masks[:, 5] = (1.0 / 3.0) * (t % 3 != 0)
    masks[:, 6] = (1.0 / 3.0) * (t % 3 != 1)
    masks[:, 7] = (1.0 / 3.0) * (t % 3 != 2)
    masks = masks.reshape(128, 8 * 32).astype(bf16)
    return gws, iot, scw, biasm, ident, masks


def kernel(input_ids, emb, conv_w, conv_b, score_w):
    global LAST_RESULT
    nc = _get_nc()
    input_ids = np.asarray(input_ids)
    emb = np.asarray(emb, dtype=np.float32)
    conv_w = np.asarray(conv_w, dtype=np.float32)
    conv_b = np.asarray(conv_b, dtype=np.float32)
    score_w = np.asarray(score_w, dtype=np.float32)
    B = input_ids.shape[0]

    gws, iot, scw, biasm, ident, masks = _host_inputs(
        input_ids, emb, conv_w, conv_b, score_w)

    # even/odd id rows, col = t+1, sentinel -7 outside [0, T)
    idsf = input_ids.astype(np.float32)
    idse = np.full((B, YW), -7.0, np.float32)
    idso = np.full((B, YW), -7.0, np.float32)
    idse[:, 1:1 + T] = idsf[:, 0::2]
    idso[:, 1:1 + T] = idsf[:, 1::2]
    idse = idse.astype(bf16)
    idso = idso.astype(bf16)

    in_maps = [{"idse": np.ascontiguousarray(idse[b:b + 1]),
                "idso": np.ascontiguousarray(idso[b:b + 1]),
                "gws": gws, "iot": iot, "scw": scw, "bias": biasm,
                "ident": ident, "mask": masks} for b in range(B)]
    res = run_bass_kernel_spmd(nc, in_maps, core_ids=list(range(B)), trace=TRACE)
    LAST_RESULT = res
    return np.stack([res.results[b]["out"] for b in range(B)]).astype(np.float32)


# revision 11
# speedup vs baseline: 1.2052x; 1.2052x over previous
"""GBST embedding kernel for Trainium2, data-parallel over batch on 8 cores.

Even/odd reformulation: with t the output index (T = L/2 after the final
DS=2 mean-pool), the whole softmax-weighted candidate combine collapses to

    o[t] = sum_{d=-2..3} w_d[t] * y[2t+d]

a 6-diagonal banded product on the t grid.  Splitting y into parity planes
yE[t]=y[2t], yO[t]=y[2t+1] makes every shifted view contiguous.

Per core (one batch element, [d_chunk, t] layout):
- Embedding gather folded into the conv: G_k = emb @ conv_w[:,:,k].T (bf16),
  y parity tiles computed as 10-matmul PSUM groups against even/odd one-hots.
- Scores s1 via scw matmuls on y tiles; softmax + w_d rows computed in
  [32-partition, 32] planes per 1024-t chunk with host-precomputed masks.
- Combine: 6 contiguous muls + 5 adds per (chunk, dc), split DVE/GPSIMD.
- PE transposes [d, t] -> [t, d] for contiguous output stores.
"""
import sys
sys.path.insert(0, "/opt/trn_rl_repo")
import numpy as np
import ml_dtypes

import concourse.bass as bass
import concourse.bacc as bacc
import concourse.tile as tile
from concourse import mybir
from concourse.bass_utils import run_bass_kernel_spmd

bf16 = ml_dtypes.bfloat16
F32 = mybir.dt.float32
BF = mybir.dt.bfloat16
OP = mybir.AluOpType
AF = mybir.ActivationFunctionType

L, T, V, D, K = 8192, 4096, 256, 512, 5
NDC, NVC = 4, 2
TT_ = 512            # conv t-tile width
NTI = T // TT_       # 8 tiles
TSEG = 1024          # chunk width in t
NCH = 4
YW = 4104            # y/oh tile width (col = t+1, t in [-1, 4103))
SW = 4104            # s1 row width
JW = 2736            # s3sum row width

TRACE = False
LAST_RESULT = None
_NC = None

# taps: (k, src_parity, col_offset_from_tile_base) for output parity E and O
TAPS = {
    "E": [(0, "E", 0), (1, "O", 0), (2, "E", 1), (3, "O", 1), (4, "E", 2)],
    "O": [(0, "O", 0), (1, "E", 1), (2, "O", 1), (3, "E", 2), (4, "O", 2)],
}
# chunk c is emitted after conv tile TRIG[c]
TRIG = {2: 0, 4: 1, 6: 2, 7: 3}


def _ceil_div(a, b):
    return -(-a // b)


def _build():
    nc = bacc.Bacc("TRN2", target_bir_lowering=False)
    idse_d = nc.dram_tensor("idse", [1, YW], BF, kind="ExternalInput")
    idso_d = nc.dram_tensor("idso", [1, YW], BF, kind="ExternalInput")
    gws_d = nc.dram_tensor("gws", [128, 40 * 128], BF, kind="ExternalInput")
    iot_d = nc.dram_tensor("iot", [128, 2], F32, kind="ExternalInput")
    scw_d = nc.dram_tensor("scw", [128, 4], BF, kind="ExternalInput")
    bias_d = nc.dram_tensor("bias", [128, 4], F32, kind="ExternalInput")
    ident_d = nc.dram_tensor("ident", [128, 128], BF, kind="ExternalInput")
    mask_d = nc.dram_tensor("mask", [128, 8 * 32], BF, kind="ExternalInput")
    out_d = nc.dram_tensor("out", [T, D], F32, kind="ExternalOutput")
    # DRAM staging for the 6 w_delta rows (broadcast source)
    wrow_d = [nc.dram_tensor(f"wrow{di}", [1, T], BF) for di in range(6)]

    with tile.TileContext(nc) as tc:
        with tc.tile_pool(name="const", bufs=1) as cst, \
             tc.tile_pool(name="persist", bufs=1) as per, \
             tc.tile_pool(name="rows", bufs=1) as rws, \
             tc.tile_pool(name="plane", bufs=1) as pln, \
             tc.tile_pool(name="wseg", bufs=1) as wsg, \
             tc.tile_pool(name="ctp", bufs=2) as ctp, \
             tc.tile_pool(name="accp", bufs=8) as accp, \
             tc.tile_pool(name="otp", bufs=2) as otp, \
             tc.tile_pool(name="psA", bufs=3, space="PSUM") as psA, \
             tc.tile_pool(name="psB", bufs=2, space="PSUM") as psB, \
             tc.tile_pool(name="psT", bufs=2, space="PSUM") as psT:

            # ---- constants
            gws_t = cst.tile([128, 40 * 128], BF)
            nc.sync.dma_start(out=gws_t[:], in_=gws_d[:])
            iot_t = cst.tile([128, 2], F32)
            nc.sync.dma_start(out=iot_t[:], in_=iot_d[:])
            scw_t = cst.tile([128, 4], BF)
            nc.sync.dma_start(out=scw_t[:], in_=scw_d[:])
            bias_t = cst.tile([128, 4], F32)
            nc.sync.dma_start(out=bias_t[:], in_=bias_d[:])
            ident_t = cst.tile([128, 128], BF)
            nc.sync.dma_start(out=ident_t[:], in_=ident_d[:])
            mask_t = cst.tile([128, 8 * 32], BF)
            nc.sync.dma_start(out=mask_t[:], in_=mask_d[:])
            MODD, MEVEN, M3E0, M3E1, M3E2, M3N0, M3N1, M3N2 = range(8)

            def mk(i):
                return mask_t[:, i * 32:(i + 1) * 32]

            # ---- persistent tensors
            ids_b = per.tile([128, YW], BF, name="idsb", tag="idsb")
            oh = {p: [per.tile([128, YW], BF, name=f"oh{p}{vc}", tag=f"oh{p}{vc}")
                      for vc in range(NVC)] for p in "EO"}
            for p, src_d in (("E", idse_d), ("O", idso_d)):
                nc.sync.dma_start(out=ids_b[:],
                                  in_=src_d[0:1, :].partition_broadcast(128))
                for vc in range(NVC):
                    nc.vector.tensor_scalar(
                        out=oh[p][vc][:], in0=ids_b[:],
                        scalar1=iot_t[:, vc:vc + 1], scalar2=None,
                        op0=OP.is_equal)

            y = {p: [per.tile([128, YW], BF, name=f"y{p}{dc}", tag=f"y{p}{dc}")
                     for dc in range(NDC)] for p in "EO"}
            for p in "EO":
                for dc in range(NDC):
                    nc.vector.memset(y[p][dc][:, 0:1], 0.0)
                    nc.vector.memset(y[p][dc][:, T + 1:YW], 0.0)
            s1 = {p: rws.tile([1, SW], BF, name=f"s1{p}", tag=f"s1{p}")
                  for p in "EO"}
            for p in "EO":
                nc.vector.memset(s1[p][0:1, T:SW], 0.0)
            s3sum = rws.tile([1, JW], BF, name="s3sum", tag="s3sum")
            us3r = {p: rws.tile([1, T + 8], BF, name=f"us3{p}", tag=f"us3{p}")
                    for p in "EO"}

            acc_tiles = {}
            ov = out_d[:].rearrange("(tb p) (dc c) -> p tb dc c", p=128, c=128)

            def emit_weights(c):
                """softmax + w_delta rows for chunk c (t in [1024c, 1024c+1024))."""
                t0 = c * TSEG
                t1 = t0 + TSEG
                P0, P1 = 32 * c, 32 * (c + 1)  # plane partition range

                # --- s3sum rows (strided, f32, DVE)
                jlo = (2 * t0) // 3
                jhi = (2 * t1 - 1) // 3
                # even j = 2m
                m0 = _ceil_div(jlo, 2)
                nme = jhi // 2 - m0 + 1
                nc.vector.tensor_tensor(
                    out=s3sum[0:1, 2 * m0:2 * m0 + 2 * nme:2],
                    in0=s1["E"][0:1, 3 * m0:3 * m0 + 3 * nme:3],
                    in1=s1["O"][0:1, 3 * m0:3 * m0 + 3 * nme:3], op=OP.add)
                nc.vector.tensor_tensor(
                    out=s3sum[0:1, 2 * m0:2 * m0 + 2 * nme:2],
                    in0=s3sum[0:1, 2 * m0:2 * m0 + 2 * nme:2],
                    in1=s1["E"][0:1, 3 * m0 + 1:3 * m0 + 1 + 3 * nme:3], op=OP.add)
                # odd j = 2m+1
                mo0 = _ceil_div(jlo - 1, 2)
                nmo = (jhi - 1) // 2 - mo0 + 1
                nc.vector.tensor_tensor(
                    out=s3sum[0:1, 2 * mo0 + 1:2 * mo0 + 1 + 2 * nmo:2],
                    in0=s1["O"][0:1, 3 * mo0 + 1:3 * mo0 + 1 + 3 * nmo:3],
                    in1=s1["E"][0:1, 3 * mo0 + 2:3 * mo0 + 2 + 3 * nmo:3], op=OP.add)
                nc.vector.tensor_tensor(
                    out=s3sum[0:1, 2 * mo0 + 1:2 * mo0 + 1 + 2 * nmo:2],
                    in0=s3sum[0:1, 2 * mo0 + 1:2 * mo0 + 1 + 2 * nmo:2],
                    in1=s1["O"][0:1, 3 * mo0 + 2:3 * mo0 + 2 + 3 * nmo:3], op=OP.add)

                # --- us3 rows: upsample s3sum/3 to the t grid (Scalar, strided)
                for side, jadd in (("E", (0, 0, 1)), ("O", (0, 1, 1))):
                    for rho in range(3):
                        ts = t0 + ((rho - t0) % 3)
                        n = _ceil_div(t1 - ts, 3)
                        mm0 = (ts - rho) // 3
                        j0 = 2 * mm0 + jadd[rho]
                        nc.scalar.activation(
                            out=us3r[side][0:1, ts:ts + 3 * n:3],
                            in_=s3sum[0:1, j0:j0 + 2 * n:2],
                            func=AF.Copy, scale=1.0 / 3.0)

                # --- softmax planes [32, 32] on partitions [P0:P1]
                LE = pln.tile([128, 32], BF, tag="LE", name=f"LE{c}")
                LO = pln.tile([128, 32], BF, tag="LO", name=f"LO{c}")
                L3a = pln.tile([128, 32], BF, tag="L3a", name=f"L3a{c}")
                L3b = pln.tile([128, 32], BF, tag="L3b", name=f"L3b{c}")
                nc.sync.dma_start(out=LE[P0:P1, :], in_=s1["E"][0:1, t0:t1])
                nc.sync.dma_start(out=LO[P0:P1, :], in_=s1["O"][0:1, t0:t1])
                nc.sync.dma_start(out=L3a[P0:P1, :], in_=us3r["E"][0:1, t0:t1])
                nc.sync.dma_start(out=L3b[P0:P1, :], in_=us3r["O"][0:1, t0:t1])
                L2 = pln.tile([128, 32], F32, tag="L2", name=f"L2{c}")
                nc.vector.tensor_tensor(out=L2[P0:P1, :], in0=LE[P0:P1, :],
                                        in1=LO[P0:P1, :], op=OP.add)
                nc.vector.tensor_scalar(out=L2[P0:P1, :], in0=L2[P0:P1, :],
                                        scalar1=0.5, scalar2=None, op0=OP.mult)
                L4 = pln.tile([128, 32], F32, tag="L4", name=f"L4{c}")
                L2pair = L2[P0:P1, :].rearrange("p (n two) -> p n two", two=2)
                nc.vector.tensor_tensor(out=L4[P0:P1, 0:16], in0=L2pair[:, :, 0],
                                        in1=L2pair[:, :, 1], op=OP.add)
                U4 = pln.tile([128, 32], F32, tag="U4", name=f"U4{c}")
                nc.vector.tensor_scalar(
                    out=U4[P0:P1, :].rearrange("p (n two) -> p n two", two=2),
                    in0=L4[P0:P1, 0:16].unsqueeze(2).to_broadcast([P1 - P0, 16, 2]),
                    scalar1=0.5, scalar2=None, op0=OP.mult)

                X = pln.tile([128, 256], BF, tag="X", name=f"X{c}")
                E = pln.tile([128, 256], BF, tag="E", name=f"E{c}")
                for half, (l1, l3) in enumerate(((LE, L3a), (LO, L3b))):
                    mx = pln.tile([128, 32], F32, tag=f"mx{half}",
                                  name=f"mx{half}_{c}")
                    nc.vector.tensor_tensor(out=mx[P0:P1, :], in0=l1[P0:P1, :],
                                            in1=L2[P0:P1, :], op=OP.max)
                    nc.vector.tensor_tensor(out=mx[P0:P1, :], in0=mx[P0:P1, :],
                                            in1=l3[P0:P1, :], op=OP.max)
                    nc.vector.tensor_tensor(out=mx[P0:P1, :], in0=mx[P0:P1, :],
                                            in1=U4[P0:P1, :], op=OP.max)
                    for li, lt in enumerate((l1, L2, l3, U4)):
                        o0 = half * 128 + li * 32
                        nc.vector.tensor_tensor(
                            out=X[P0:P1, o0:o0 + 32], in0=lt[P0:P1, :],
                            in1=mx[P0:P1, :], op=OP.subtract)
                nc.scalar.activation(out=E[P0:P1, :], in_=X[P0:P1, :], func=AF.Exp)

                W = {}
                for half, pname in enumerate("EO"):
                    h0 = half * 128
                    Z = pln.tile([128, 32], F32, tag=f"Z{half}",
                                 name=f"Z{half}_{c}")
                    nc.vector.tensor_tensor(out=Z[P0:P1, :],
                                            in0=E[P0:P1, h0:h0 + 32],
                                            in1=E[P0:P1, h0 + 32:h0 + 64], op=OP.add)
                    nc.vector.tensor_tensor(out=Z[P0:P1, :], in0=Z[P0:P1, :],
                                            in1=E[P0:P1, h0 + 64:h0 + 96], op=OP.add)
                    nc.vector.tensor_tensor(out=Z[P0:P1, :], in0=Z[P0:P1, :],
                                            in1=E[P0:P1, h0 + 96:h0 + 128], op=OP.add)
                    R = pln.tile([128, 32], F32, tag=f"R{half}",
                                 name=f"R{half}_{c}")
                    nc.vector.reciprocal(out=R[P0:P1, :], in_=Z[P0:P1, :])
                    # R' = 0.5/Z
                    nc.vector.tensor_scalar(out=R[P0:P1, :], in0=R[P0:P1, :],
                                            scalar1=0.5, scalar2=None, op0=OP.mult)
                    for mi in range(4):
                        wt = pln.tile([128, 32], F32, tag=f"w{mi}{half}",
                                      name=f"w{mi}{half}_{c}")
                        nc.vector.tensor_tensor(
                            out=wt[P0:P1, :],
                            in0=E[P0:P1, h0 + mi * 32:h0 + mi * 32 + 32],
                            in1=R[P0:P1, :], op=OP.mult)
                        W[(mi, pname)] = wt

                # combined raw sums (already carry the 0.5 from R')
                W2s = pln.tile([128, 32], F32, tag="W2s", name=f"W2s{c}")
                nc.vector.tensor_tensor(out=W2s[P0:P1, :], in0=W[(1, "E")][P0:P1, :],
                                        in1=W[(1, "O")][P0:P1, :], op=OP.add)
                W4s = pln.tile([128, 32], F32, tag="W4s", name=f"W4s{c}")
                nc.vector.tensor_tensor(out=W4s[P0:P1, :], in0=W[(3, "E")][P0:P1, :],
                                        in1=W[(3, "O")][P0:P1, :], op=OP.add)
                P2 = pln.tile([128, 32], F32, tag="P2", name=f"P2{c}")
                nc.vector.tensor_scalar(out=P2[P0:P1, :], in0=W2s[P0:P1, :],
                                        scalar1=0.5, scalar2=None, op0=OP.mult)
                P4d = pln.tile([128, 32], F32, tag="P4d", name=f"P4d{c}")
                nc.vector.tensor_scalar(out=P4d[P0:P1, :], in0=W4s[P0:P1, :],
                                        scalar1=0.25, scalar2=None, op0=OP.mult)
                P4o = pln.tile([128, 32], F32, tag="P4o", name=f"P4o{c}")
                nc.vector.tensor_tensor(out=P4o[P0:P1, :], in0=W4s[P0:P1, :],
                                        in1=mk(MODD)[P0:P1, :], op=OP.mult)
                P4e = pln.tile([128, 32], F32, tag="P4e", name=f"P4e{c}")
                nc.vector.tensor_tensor(out=P4e[P0:P1, :], in0=W4s[P0:P1, :],
                                        in1=mk(MEVEN)[P0:P1, :], op=OP.mult)

                w3E, w3O = W[(2, "E")], W[(2, "O")]
                q = {}
                for nm, src, mi in (("a_r2", w3E, M3E1), ("a_n0", w3E, M3N0),
                                    ("a_n2", w3E, M3N1), ("a_r0", w3E, M3E0),
                                    ("b_e2", w3O, M3E2), ("b_ne1", w3O, M3N1),
                                    ("b_ne2", w3O, M3N2), ("b_e1", w3O, M3E1)):
                    qt = pln.tile([128, 32], F32, tag=f"q{nm}", name=f"q{nm}_{c}")
                    nc.vector.tensor_tensor(out=qt[P0:P1, :], in0=src[P0:P1, :],
                                            in1=mk(mi)[P0:P1, :], op=OP.mult)
                    q[nm] = qt
                qa_d = pln.tile([128, 32], F32, tag="qa_d", name=f"qa_d{c}")
                nc.vector.tensor_scalar(out=qa_d[P0:P1, :], in0=w3E[P0:P1, :],
                                        scalar1=1.0 / 3.0, scalar2=None, op0=OP.mult)
                qb_d = pln.tile([128, 32], F32, tag="qb_d", name=f"qb_d{c}")
                nc.vector.tensor_scalar(out=qb_d[P0:P1, :], in0=w3O[P0:P1, :],
                                        scalar1=1.0 / 3.0, scalar2=None, op0=OP.mult)

                wd = [pln.tile([128, 32], BF, tag=f"wd{di}", name=f"wd{di}_{c}")
                      for di in range(6)]
                # delta=-2: P4o + qa_r2
                nc.vector.tensor_tensor(out=wd[0][P0:P1, :], in0=P4o[P0:P1, :],
                                        in1=q["a_r2"][P0:P1, :], op=OP.add)
                # delta=-1: P4o + qa_n0 + qb_e2
                tmp = pln.tile([128, 32], F32, tag="wtmp", name=f"wtmp{c}")
                nc.vector.tensor_tensor(out=tmp[P0:P1, :], in0=P4o[P0:P1, :],
                                        in1=q["a_n0"][P0:P1, :], op=OP.add)
                nc.vector.tensor_tensor(out=wd[1][P0:P1, :], in0=tmp[P0:P1, :],
                                        in1=q["b_e2"][P0:P1, :], op=OP.add)
                # delta=0: w1E' + P2 + P4d + qa_d + qb_ne1
                t0_ = pln.tile([128, 32], F32, tag="wt0", name=f"wt0_{c}")
                nc.vector.tensor_tensor(out=t0_[P0:P1, :],
                                        in0=W[(0, "E")][P0:P1, :],
                                        in1=P2[P0:P1, :], op=OP.add)
                nc.vector.tensor_tensor(out=t0_[P0:P1, :], in0=t0_[P0:P1, :],
                                        in1=P4d[P0:P1, :], op=OP.add)
                nc.vector.tensor_tensor(out=t0_[P0:P1, :], in0=t0_[P0:P1, :],
                                        in1=qa_d[P0:P1, :], op=OP.add)
                nc.vector.tensor_tensor(out=wd[2][P0:P1, :], in0=t0_[P0:P1, :],
                                        in1=q["b_ne1"][P0:P1, :], op=OP.add)
                # delta=1: w1O' + P2 + P4d + qa_n2 + qb_d
                t1_ = pln.tile([128, 32], F32, tag="wt1", name=f"wt1_{c}")
                nc.vector.tensor_tensor(out=t1_[P0:P1, :],
                                        in0=W[(0, "O")][P0:P1, :],
                                        in1=P2[P0:P1, :], op=OP.add)
                nc.vector.tensor_tensor(out=t1_[P0:P1, :], in0=t1_[P0:P1, :],
                                        in1=P4d[P0:P1, :], op=OP.add)
                nc.vector.tensor_tensor(out=t1_[P0:P1, :], in0=t1_[P0:P1, :],
                                        in1=q["a_n2"][P0:P1, :], op=OP.add)
                nc.vector.tensor_tensor(out=wd[3][P0:P1, :], in0=t1_[P0:P1, :],
                                        in1=qb_d[P0:P1, :], op=OP.add)
                # delta=2: P4e + qa_r0 + qb_ne2
                tmp2 = pln.tile([128, 32], F32, tag="wtmp2", name=f"wtmp2{c}")
                nc.vector.tensor_tensor(out=tmp2[P0:P1, :], in0=P4e[P0:P1, :],
                                        in1=q["a_r0"][P0:P1, :], op=OP.add)
                nc.vector.tensor_tensor(out=wd[4][P0:P1, :], in0=tmp2[P0:P1, :],
                                        in1=q["b_ne2"][P0:P1, :], op=OP.add)
                # delta=3: P4e + qb_e1
                nc.vector.tensor_tensor(out=wd[5][P0:P1, :], in0=P4e[P0:P1, :],
                                        in1=q["b_e1"][P0:P1, :], op=OP.add)

                for di in range(6):
                    nc.sync.dma_start(out=wrow_d[di][0:1, t0:t1],
                                      in_=wd[di][P0:P1, :])

            def emit_combine(c):
                t0 = c * TSEG
                wb = []
                for di in range(6):
                    wt = wsg.tile([128, TSEG], BF, tag=f"wb{di}",
                                  name=f"wb{di}_{c}")
                    nc.sync.dma_start(
                        out=wt[:],
                        in_=wrow_d[di][0:1, t0:t0 + TSEG].partition_broadcast(128))
                    wb.append(wt)
                for dc in range(NDC):
                    yE, yO = y["E"][dc], y["O"][dc]
                    # views: col = t+1
                    vEm2 = yE[:, t0:t0 + TSEG]          # yE[t-1], aligned
                    vE0 = yE[:, t0 + 1:t0 + 1 + TSEG]   # yE[t], misaligned
                    vE2 = yE[:, t0 + 2:t0 + 2 + TSEG]   # yE[t+1], aligned
                    vOm1 = yO[:, t0:t0 + TSEG]
                    vO1 = yO[:, t0 + 1:t0 + 1 + TSEG]
                    vO3 = yO[:, t0 + 2:t0 + 2 + TSEG]
                    # all-DVE chain (GPSIMD shares the SBUF port with DVE --
                    # co-running them makes both ~4x slower)
                    acc = accp.tile([128, TSEG], BF, tag="acc",
                                    name=f"acc_{c}_{dc}")
                    tD = ctp.tile([128, TSEG], BF, tag="tD", name=f"tD_{c}_{dc}")
                    nc.vector.tensor_tensor(out=acc[:], in0=vEm2, in1=wb[0][:],
                                            op=OP.mult)
                    nc.vector.tensor_tensor(out=tD[:], in0=vOm1, in1=wb[1][:],
                                            op=OP.mult)
                    nc.vector.tensor_tensor(out=acc[:], in0=acc[:], in1=tD[:],
                                            op=OP.add)
                    nc.vector.tensor_tensor(out=tD[:], in0=vE2, in1=wb[4][:],
                                            op=OP.mult)
                    nc.vector.tensor_tensor(out=acc[:], in0=acc[:], in1=tD[:],
                                            op=OP.add)
                    nc.vector.tensor_tensor(out=tD[:], in0=vO3, in1=wb[5][:],
                                            op=OP.mult)
                    nc.vector.tensor_tensor(out=acc[:], in0=acc[:], in1=tD[:],
                                            op=OP.add)
                    nc.vector.tensor_tensor(out=tD[:], in0=vE0, in1=wb[2][:],
                                            op=OP.mult)
                    nc.vector.tensor_tensor(out=acc[:], in0=acc[:], in1=tD[:],
                                            op=OP.add)
                    nc.vector.tensor_tensor(out=tD[:], in0=vO1, in1=wb[3][:],
                                            op=OP.mult)
                    nc.vector.tensor_tensor(out=acc[:], in0=acc[:], in1=tD[:],
                                            op=OP.add)
                    acc_tiles[(c, dc)] = acc

            def emit_tr(c):
                """PE transposes + stores for chunk c (deferred so the PE
                FIFO doesn't stall on the DVE combine)."""
                for dc in range(NDC):
                    acc = acc_tiles.pop((c, dc))
                    for q4 in range(2):
                        pt = psT.tile([128, 512], BF, tag="tp",
                                      name=f"pt_{c}_{dc}_{q4}")
                        for qq in range(4):
                            nc.tensor.transpose(
                                out=pt[:, qq * 128:(qq + 1) * 128],
                                in_=acc[:, (q4 * 4 + qq) * 128:
                                        (q4 * 4 + qq + 1) * 128],
                                identity=ident_t[:])
                        ot = otp.tile([128, 512], F32, tag="ot",
                                      name=f"ot_{c}_{dc}_{q4}")
                        nc.scalar.copy(out=ot[:], in_=pt[:])
                        tb0 = c * 8 + q4 * 4
                        nc.sync.dma_start(
                            out=ov[:, tb0:tb0 + 4, dc, :],
                            in_=ot[:].rearrange("p (tb c) -> p tb c", c=128))

            # ---- main conv loop
            def emit_conv(i):
                col0 = i * TT_
                for p in "EO":
                    for dc in range(NDC):
                        ps = psA.tile([128, TT_], F32, tag="convps",
                                      name=f"ps_{i}_{p}_{dc}")
                        for j, (k, src, off) in enumerate(TAPS[p]):
                            for vc in range(NVC):
                                jj = j * 2 + vc
                                nc.tensor.matmul(
                                    out=ps[:],
                                    lhsT=gws_t[:, ((k * 2 + vc) * 4 + dc) * 128:
                                               ((k * 2 + vc) * 4 + dc) * 128 + 128],
                                    rhs=oh[src][vc][:, col0 + off:
                                                    col0 + off + TT_],
                                    start=(jj == 0), stop=(jj == 9))
                        nc.scalar.activation(
                            out=y[p][dc][:, col0 + 1:col0 + 1 + TT_], in_=ps[:],
                            func=AF.Identity, bias=bias_t[:, dc:dc + 1])

            def emit_s1(i):
                col0 = i * TT_
                for p in "EO":
                    ps1 = psB.tile([1, TT_], F32, tag="s1ps", name=f"ps1_{i}_{p}")
                    for dc in range(NDC):
                        nc.tensor.matmul(out=ps1[:], lhsT=scw_t[:, dc:dc + 1],
                                         rhs=y[p][dc][:, col0 + 1:col0 + 1 + TT_],
                                         start=(dc == 0), stop=(dc == NDC - 1))
                    nc.scalar.copy(out=s1[p][0:1, col0:col0 + TT_], in_=ps1[:])

            # s1 for tile j is emitted one conv tile late (its dequants finish
            # during tile j+1's conv, so the PE FIFO never stalls); chunk
            # weights+combine (non-PE work) are emitted as soon as their s1
            # span is complete; transposes (PE) are deferred ~2 conv tiles.
            for i in range(NTI):
                emit_conv(i)
                if i >= 1:
                    emit_s1(i - 1)
                if i == 3:
                    emit_weights(0)
                    emit_combine(0)
                elif i == 5:
                    emit_weights(1)
                    emit_combine(1)
                    emit_tr(0)
                elif i == 7:
                    emit_weights(2)
                    emit_combine(2)
                    emit_s1(7)
                    emit_weights(3)
                    emit_combine(3)
                    emit_tr(1)
                    emit_tr(2)
                    emit_tr(3)
    nc.compile()
    return nc


def _get_nc():
    global _NC
    if _NC is None:
        _NC = _build()
    return _NC


def _host_inputs(input_ids, emb, conv_w, conv_b, score_w):
    G = np.einsum("oik,vi->kvo", conv_w.astype(np.float64),
                  emb.astype(np.float64)).astype(np.float32)  # [K, V, D]
    gws = np.zeros((128, 40, 128), np.float32)
    for k in range(K):
        for vc in range(NVC):
            for dc in range(NDC):
                gws[:, (k * 2 + vc) * 4 + dc, :] = \
                    G[k, vc * 128:(vc + 1) * 128, dc * 128:(dc + 1) * 128]
    gws = gws.reshape(128, 40 * 128).astype(bf16)
    iot = np.stack([np.arange(128), np.arange(128) + 128], axis=1).astype(np.float32)
    scw = score_w.reshape(4, 128).T.astype(bf16)
    biasm = conv_b.reshape(4, 128).T.astype(np.float32)
    ident = np.eye(128, dtype=np.float32).astype(bf16)

    t = (np.arange(128)[:, None] * 32 + np.arange(32)[None, :])  # [128, 32]
    masks = np.zeros((128, 8, 32), np.float32)
    masks[:, 0] = 0.25 * (t % 2 == 1)
    masks[:, 1] = 0.25 * (t % 2 == 0)
    masks[:, 2] = (1.0 / 3.0) * (t % 3 == 0)
    masks[:, 3] = (1.0 / 3.0) * (t % 3 == 1)
    masks[:, 4] = (1.0 / 3.0) * (t % 3 == 2)
    masks[:, 5] = (1.0 / 3.0) * (t % 3 != 0)
    masks[:, 6] = (1.0 / 3.0) * (t % 3 != 1)
    masks[:, 7] = (1.0 / 3.0) * (t % 3 != 2)
    masks = masks.reshape(128, 8 * 32).astype(bf16)
    return gws, iot, scw, biasm, ident, masks


def kernel(input_ids, emb, conv_w, conv_b, score_w):
    global LAST_RESULT
    nc = _get_nc()
    input_ids = np.asarray(input_ids)
    emb = np.asarray(emb, dtype=np.float32)
    conv_w = np.asarray(conv_w, dtype=np.float32)
    conv_b = np.asarray(conv_b, dtype=np.float32)
    score_w = np.asarray(score_w, dtype=np.float32)
    B = input_ids.shape[0]

    gws, iot, scw, biasm, ident, masks = _host_inputs(
        input_ids, emb, conv_w, conv_b, score_w)

    # even/odd id rows, col = t+1, sentinel -7 outside [0, T)
    idsf = input_ids.astype(np.float32)
    idse = np.full((B, YW), -7.0, np.float32)
    idso = np.full((B, YW), -7.0, np.float32)
    idse[:, 1:1 + T] = idsf[:, 0::2]
    idso[:, 1:1 + T] = idsf[:, 1::2]
    idse = idse.astype(bf16)
    idso = idso.astype(bf16)

    in_maps = [{"idse": np.ascontiguousarray(idse[b:b + 1]),
                "idso": np.ascontiguousarray(idso[b:b + 1]),
                "gws": gws, "iot": iot, "scw": scw, "bias": biasm,
                "ident": ident, "mask": masks} for b in range(B)]
    res = run_bass_kernel_spmd(nc, in_maps, core_ids=list(range(B)), trace=TRACE)
    LAST_RESULT = res
    return np.stack([res.results[b]["out"] for b in range(B)]).astype(np.float32)
